# revision 2
# baseline (speedup 1.0000x reference)
"""Multi-head attention kernel for 8 TRN2 NeuronCores (v2).

Problem: B=2, S=2048, D=1024, H=16 heads (HD=64).
  q/k/v = x @ W{q,k,v}.T + b;  out = softmax(q k^T / 8 + mask) v  (heads merged)

Sharding: core c owns batch b = c // 4 and head group g = c % 4 (4 heads,
256 channels).  No collectives: each core computes its [2048, 256] slice of
the output; the host gathers.

Changes vs the 221 us baseline (TimelineSim 205 -> 172.6 us; measured
~199 us in the same session where the baseline re-measured 226 us, a
~12% speedup; all per-core PE matmul streams sit at the dataflow
roofline of 360448 moving rows = 150 us @ 2.4 GHz):
  - V bias via DVE tensor_add against a host-broadcast bias tile
    (drops the 16 ones-row bias matmuls).
  - All PSUM pools coexist (psS 2x2 banks + psPH 2x1 + psT 2x1 = 8
    banks): no mid-kernel pool-scope transition barrier (was ~6 us).
  - DMA order follows consumption order (wq, xt s-chunk 0 split in two,
    wk, biases/mask, xt 1-3, wv) so the first matmul starts ~4.5 us in.
  - One globally paced emission schedule replaces the phase windows: a
    B1 stream (the exp feed, halves-major per head) is interleaved with
    "filler" PE quanta (projection chunks, V tiles, B2 chunks) drawn
    greedily in dependency order against a rows-proportional pace, with
    per-filler eligibility (B2 chunk (h,qp,kc) unlocks 5 B1 items after
    the E halves it reads) and deadlines (K chunk c before B1 kt=4c).
    The PE stream stays dense (p-state stays at 2.4 GHz) and ACT always
    has an exp pending; B2 work drifts into the tail head's window.
  - B2 pair-chunks run kt-outer over a qc pair so each V_aug stationary
    load covers 2 matmuls (fewer PE weight reloads; measured win).

Kept from the baseline (measured ~20 us better than the "cheaper"
K=64 alternative): Q^T/K^T duplicated into both partition halves so
score matmuls contract over K=128; the 2x score is absorbed into the
exp scale (0.0625).  Real HW runs K=64 matmuls well below the cost
model's free-dim-only estimate.

Per-core dataflow (operands fp16, accumulation fp32 in PSUM):
  proj single (et, qc): psPH[128,512] = sum_d Wq/Wk[d,et]^T x^T[d,qc];
    DVE adds bias and writes per-head duplicated qt/kt [128, S] tiles.
  V quantum (st): psPH[128,256] = sum_d x^T[d,st]^T Wv[d]; DVE writes
    V_aug[st] [128, 4*65] fp16 (+bias, ones column per head).
  B1 group (h, kt, half): psS[128,1024] = K^T[kt]^T Q^T (2 matmuls),
    ACT exp(0.0625 s + mask[kt]) -> E half tile [128, 1024] fp16.
  B2 pair-chunk (h, qp, kc): psT[65, 512] (x2, qc = 2qp, 2qp+1) +=
    V_aug[kt]^T E, kt-outer over 4 kt; at kc=3 DVE copies both to SBUF
    and DMA writes ctx^T[h] (row 64 = softmax denominator; the host
    divides and transposes).
"""

import numpy as np

import concourse.mybir as mybir
import concourse.tile as tile
from concourse import bacc
from concourse.bass_utils import run_bass_kernel_spmd

B, S, D, H = 2, 2048, 1024, 16
HD = D // H           # 64
EPC = 256             # e-channels per core (4 heads)
HPC = 4               # heads per core
NDT = D // 128        # 8 d tiles
NKT = S // 128        # 16 k tiles
W = 1024              # exp width (psum tile [128, W], 2 banks)

f32 = mybir.dt.float32
fp16 = mybir.dt.float16

_CACHE = {}


def _build(reps=1, trace_sim=False, loop=False, cache=True):
    key = (reps, loop)
    if cache and key in _CACHE:
        return _CACHE[key]

    nc = bacc.Bacc("TRN2", target_bir_lowering=False, debug=False, num_devices=8)

    # xt is laid out [128, NDT*S] host-side so DMAs are d-contiguous per s-chunk
    xt_d = nc.dram_tensor("xt", [128, NDT * S], fp16, kind="ExternalInput")
    wq_d = nc.dram_tensor("wq", [128, NDT * EPC], fp16, kind="ExternalInput")
    wk_d = nc.dram_tensor("wk", [128, NDT * EPC], fp16, kind="ExternalInput")
    wv_d = nc.dram_tensor("wv", [128, NDT * EPC], fp16, kind="ExternalInput")
    bq_d = nc.dram_tensor("bq", [128, 2], f32, kind="ExternalInput")
    bk_d = nc.dram_tensor("bk", [128, 2], f32, kind="ExternalInput")
    bvb_d = nc.dram_tensor("bvb", [128, EPC], f32, kind="ExternalInput")
    mask_d = nc.dram_tensor("mask", [128, NKT], f32, kind="ExternalInput")
    ctxt_d = nc.dram_tensor("ctxt", [HPC, HD + 1, S], f32, kind="ExternalOutput")

    Exp = mybir.ActivationFunctionType.Exp

    with tile.TileContext(nc, trace_sim=trace_sim) as tc:
        with tc.tile_pool(name="sb", bufs=1) as sb, \
             tc.tile_pool(name="wpool", bufs=1) as wpool, \
             tc.tile_pool(name="xpool", bufs=1) as xpool, \
             tc.tile_pool(name="epool", bufs=52) as epool, \
             tc.tile_pool(name="cout", bufs=4) as cout, \
             tc.tile_pool(name="psS", bufs=2, space="PSUM") as psS, \
             tc.tile_pool(name="psPH", bufs=2, space="PSUM") as psPH, \
             tc.tile_pool(name="psT", bufs=2, space="PSUM") as psT:

            # ---- loads (DMA queue order == consumption order) ----
            bq_sb = sb.tile([128, 2], f32, tag="bq")
            bk_sb = sb.tile([128, 2], f32, tag="bk")
            mask_sb = sb.tile([128, NKT], f32, tag="mask")
            bvb_sb = sb.tile([128, EPC], f32, tag="bvb")
            bvb_r = bvb_sb.rearrange("p (h c) -> p h c", h=HPC)

            wq_sb, wk_sb, wv_sb = [], [], []
            bigs = {}
            for name, lst in (("wq", wq_sb), ("wk", wk_sb), ("wv", wv_sb)):
                big = wpool.tile([128, NDT * EPC], fp16, tag=name, name=name)
                bigs[name] = big
                lst.extend(big[:, d * EPC:(d + 1) * EPC] for d in range(NDT))

            xt_big = xpool.tile([128, NDT * S], fp16, tag="xt", name="xt")
            xt_r = xt_big.rearrange("p (d s) -> p d s", d=NDT)
            xtd_r = xt_d.rearrange("p (d s) -> p d s", d=NDT)
            xt_sb = [xt_big[:, d * S:(d + 1) * S] for d in range(NDT)]

            nc.sync.dma_start(out=bigs["wq"], in_=wq_d[:, :])
            nc.sync.dma_start(out=xt_r[:, 0:4, 0:512], in_=xtd_r[:, 0:4, 0:512])
            nc.sync.dma_start(out=xt_r[:, 4:8, 0:512], in_=xtd_r[:, 4:8, 0:512])
            nc.sync.dma_start(out=bigs["wk"], in_=wk_d[:, :])
            nc.sync.dma_start(out=bq_sb, in_=bq_d[:, :])
            nc.sync.dma_start(out=bk_sb, in_=bk_d[:, :])
            nc.sync.dma_start(out=mask_sb, in_=mask_d[:, :])
            nc.sync.dma_start(out=bvb_sb, in_=bvb_d[:, :])
            for sc in range(1, 4):
                nc.sync.dma_start(
                    out=xt_r[:, :, sc * 512:(sc + 1) * 512],
                    in_=xtd_r[:, :, sc * 512:(sc + 1) * 512],
                )
            nc.sync.dma_start(out=bigs["wv"], in_=wv_d[:, :])

            # Q^T/K^T per-head tiles [128, S]: head's 64 rows duplicated into
            # both partition halves (contraction 128 at full SBUF bandwidth;
            # scores double, absorbed into the exp scale)
            qt_sb = [sb.tile([128, S], fp16, tag=f"qt{h}", name=f"qt{h}")
                     for h in range(HPC)]
            kt_sb = [sb.tile([128, S], fp16, tag=f"kt{h}", name=f"kt{h}")
                     for h in range(HPC)]
            vaug = [sb.tile([128, HPC * (HD + 1)], fp16, tag=f"vaug{st}",
                            name=f"vaug{st}")
                    for st in range(NKT)]

            def emit_body(tc, rep):
                e_map = {}
                pst_map = {}

                def proj_epilogue(ps, b_sb, dsts, et, qc):
                    cs = slice(qc * 512, (qc + 1) * 512)
                    for i, h in enumerate((2 * et, 2 * et + 1)):
                        src = ps[i * 64:(i + 1) * 64, :]
                        bias = b_sb[i * 64:(i + 1) * 64, et:et + 1]
                        nc.vector.tensor_scalar_add(dsts[h][0:64, cs], src, bias)
                        nc.vector.tensor_scalar_add(dsts[h][64:128, cs], src, bias)

                def proj_single(w_lst, b_sb, dsts, et, qc):
                    """One qc (startup only: needs just one xt s-chunk)."""
                    ps = psPH.tile([128, 512], f32, tag="ph", name="ph")
                    for d in range(NDT):
                        nc.tensor.matmul(
                            ps,
                            w_lst[d][:, et * 128:(et + 1) * 128],
                            xt_sb[d][:, qc * 512:(qc + 1) * 512],
                            start=(d == 0), stop=(d == NDT - 1),
                        )
                    proj_epilogue(ps, b_sb, dsts, et, qc)

                def v_quantum(st):
                    ps = psPH.tile([128, EPC], f32, tag="ph", name="ph")
                    for d in range(NDT):
                        nc.tensor.matmul(
                            ps, xt_sb[d][:, st * 128:(st + 1) * 128], wv_sb[d],
                            start=(d == 0), stop=(d == NDT - 1),
                        )
                    vt = vaug[st].rearrange("p (h c) -> p h c", h=HPC)
                    nc.vector.memset(vt[:, :, HD:HD + 1], 1.0)
                    nc.vector.tensor_add(
                        vt[:, :, 0:HD],
                        ps.rearrange("p (h c) -> p h c", h=HPC),
                        bvb_r,
                    )

                def b1_group(h, kt, half):
                    e_t = epool.tile([128, W], fp16, tag="E", name="E")
                    e_map[(h, kt, half)] = e_t
                    ps = psS.tile([128, W], f32, tag="psS", name="psS")
                    for j in range(2):
                        qc = half * 2 + j
                        nc.tensor.matmul(
                            ps[:, j * 512:(j + 1) * 512],
                            kt_sb[h][:, kt * 128:(kt + 1) * 128],
                            qt_sb[h][:, qc * 512:(qc + 1) * 512],
                            start=True, stop=True,
                        )
                    # inputs duplicated across partition halves -> raw scores
                    # are 2x -> scale 1/(2*sqrt(HD))
                    nc.scalar.activation(
                        e_t, ps, Exp,
                        bias=mask_sb[:, kt:kt + 1], scale=0.0625,
                    )

                def b2_chunk_pair(h, qp, kc):
                    """8 matmuls: kt-outer, qc-pair-inner, so each vaug[kt]
                    stationary load covers 2 consecutive matmuls."""
                    qcs = (2 * qp, 2 * qp + 1)
                    if kc == 0:
                        for qc in qcs:
                            pst_map[(h, qc)] = psT.tile(
                                [HD + 1, 512], f32, tag="psT", name="psT")
                    for kt in range(4 * kc, 4 * kc + 4):
                        e_t = e_map[(h, kt, qp)]
                        for j, qc in enumerate(qcs):
                            nc.tensor.matmul(
                                pst_map[(h, qc)],
                                vaug[kt][:, h * (HD + 1):(h + 1) * (HD + 1)],
                                e_t[:, j * 512:(j + 1) * 512],
                                start=(kt == 0), stop=(kt == NKT - 1),
                            )
                    if kc == 3:
                        for qc in qcs:
                            o = cout.tile([HD + 1, 512], f32, tag="cout",
                                          name="cout")
                            nc.vector.tensor_copy(o, pst_map[(h, qc)])
                            nc.sync.dma_start(
                                out=ctxt_d[h, :, qc * 512:(qc + 1) * 512], in_=o)
                            del pst_map[(h, qc)]

                # ---- globally paced schedule ----
                # B1 stream items: per head, all half-0 groups then all
                # half-1 (halves-major: the first exp fires early and B2
                # qc-pairs unlock at mid-head).
                b1_items = []
                for h in range(HPC):
                    for half in range(2):
                        for kt in range(NKT):
                            b1_items.append((1024, [(h, kt, half)]))

                # Fillers: (rows, eligible_idx, deadline_idx, thunk), emitted
                # in list order.  eligible: not before b1_items[eligible] has
                # been emitted; deadline: must be emitted before
                # b1_items[deadline].
                INF = 10 ** 9
                fillers = []

                # rest of e-tile 0 as singles (startup is DMA-paced; a
                # coarser quantum would stall on two xt s-chunks at once).
                # K chunk c feeds B1 kt 4c..4c+3; Q chunks 2,3 feed the
                # half-1 items.
                def ps1(w, b, dst, et, qc, dl):
                    fillers.append((4096, 0, dl,
                                    lambda: proj_single(w, b, dst, et, qc)))

                ps1(wk_sb, bk_sb, kt_sb, 0, 1, 4)
                ps1(wk_sb, bk_sb, kt_sb, 0, 2, 8)
                ps1(wk_sb, bk_sb, kt_sb, 0, 3, 12)
                ps1(wq_sb, bq_sb, qt_sb, 0, 2, 16)
                ps1(wq_sb, bq_sb, qt_sb, 0, 3, 16)
                # V (list order keeps all V before the first B2 chunk)
                for st in range(NKT):
                    fillers.append((2048, 0, 17 + st,
                                    lambda st=st: v_quantum(st)))
                # B2 h0 pair-chunks: E halves (0, kt<=4kc+3, qp) ready after
                # b1 item 16*qp + 4kc + 3
                for qp in range(2):
                    for kc in range(4):
                        el = 16 * qp + 4 * kc + 4
                        fillers.append(
                            (4096, el, INF,
                             lambda qp=qp, kc=kc: b2_chunk_pair(0, qp, kc)))
                # e-tile 1 (heads 2, 3): needed progressively by B1 h2
                # (items 64-95); singles keep the exp cadence smooth
                ps1(wq_sb, bq_sb, qt_sb, 1, 0, 64)
                ps1(wk_sb, bk_sb, kt_sb, 1, 0, 64)
                ps1(wq_sb, bq_sb, qt_sb, 1, 1, 64)
                ps1(wk_sb, bk_sb, kt_sb, 1, 1, 68)
                ps1(wk_sb, bk_sb, kt_sb, 1, 2, 72)
                ps1(wk_sb, bk_sb, kt_sb, 1, 3, 76)
                ps1(wq_sb, bq_sb, qt_sb, 1, 2, 80)
                ps1(wq_sb, bq_sb, qt_sb, 1, 3, 80)
                # B2 h1-3: chunk (h, qp, kc) eligible after b1 item
                # 32h + 16*qp + 4kc + 3
                for h in (1, 2, 3):
                    for qp in range(2):
                        for kc in range(4):
                            el = 32 * h + 16 * qp + 4 * kc + 4
                            fillers.append(
                                (4096, el, INF,
                                 lambda h=h, qp=qp, kc=kc: b2_chunk_pair(h, qp, kc)))

                # prologue: enough of Q/K e-tile 0 for B1 h0 to start, in
                # single-qc quanta so the first matmul only needs xt chunk 0
                proj_single(wq_sb, bq_sb, qt_sb, 0, 0)
                proj_single(wk_sb, bk_sb, kt_sb, 0, 0)
                proj_single(wq_sb, bq_sb, qt_sb, 0, 1)

                total_rows = sum(f[0] for f in fillers)
                b1_rows = sum(r for r, _ in b1_items)
                ratio = total_rows / b1_rows
                acc = 0.0
                fi = 0
                for i, (rows, groups) in enumerate(b1_items):
                    while fi < len(fillers) and fillers[fi][2] <= i:
                        fillers[fi][3]()
                        acc -= fillers[fi][0]
                        fi += 1
                    acc += rows * ratio
                    while (fi < len(fillers) and fillers[fi][1] <= i
                           and fillers[fi][0] <= acc):
                        fillers[fi][3]()
                        acc -= fillers[fi][0]
                        fi += 1
                    for h, kt, half in groups:
                        b1_group(h, kt, half)
                while fi < len(fillers):
                    fillers[fi][3]()
                    fi += 1

            if loop and reps > 1:
                with tc.For_i(0, reps, 1):
                    emit_body(tc, 0)
            else:
                for rep in range(reps):
                    emit_body(tc, rep)

    nc.compile()
    if cache:
        _CACHE[key] = nc
    return nc


def _wlayout(w):
    """[EPC, D] nn.Linear weight slice -> [128, NDT*EPC] (d-tiles along free dim)."""
    wt = w.T.astype(np.float16).reshape(NDT, 128, EPC)
    return np.ascontiguousarray(wt.transpose(1, 0, 2).reshape(128, NDT * EPC))


def _prep_inputs(hidden_states, attn_mask, Wq, bq, Wk, bk, Wv, bv):
    """Build the 8 per-core input maps (host-side sharding)."""
    in_maps = []
    xt_b = {}
    for b in range(B):
        xt = hidden_states[b].T.astype(np.float16).reshape(NDT, 128, S)  # [D, S] tiled
        xt_b[b] = np.ascontiguousarray(xt.transpose(1, 0, 2).reshape(128, NDT * S))
    mask_b = {
        b: np.ascontiguousarray(
            np.asarray(attn_mask[b, 0, 0, :], dtype=np.float32).reshape(NKT, 128).T
        )
        for b in range(B)
    }
    for c in range(8):
        b, g = divmod(c, HPC)
        sl = slice(g * EPC, (g + 1) * EPC)
        in_maps.append({
            "xt": xt_b[b],
            "wq": _wlayout(Wq[sl, :]),
            "wk": _wlayout(Wk[sl, :]),
            "wv": _wlayout(Wv[sl, :]),
            "bq": np.ascontiguousarray(np.asarray(bq[sl], np.float32).reshape(2, 128).T),
            "bk": np.ascontiguousarray(np.asarray(bk[sl], np.float32).reshape(2, 128).T),
            "bvb": np.ascontiguousarray(
                np.tile(np.asarray(bv[sl], np.float32).reshape(1, EPC), (128, 1))),
            "mask": mask_b[b],
        })
    return in_maps


def kernel(hidden_states, attn_mask, Wq, bq, Wk, bk, Wv, bv):
    hidden_states = np.asarray(hidden_states)
    attn_mask = np.asarray(attn_mask)
    Wq, bq = np.asarray(Wq), np.asarray(bq)
    Wk, bk = np.asarray(Wk), np.asarray(bk)
    Wv, bv = np.asarray(Wv), np.asarray(bv)

    nc = _build()
    in_maps = _prep_inputs(hidden_states, attn_mask, Wq, bq, Wk, bk, Wv, bv)
    res = run_bass_kernel_spmd(nc, in_maps, list(range(8)))

    out = np.empty((B, S, D), np.float32)
    for c in range(8):
        b, g = divmod(c, HPC)
        ctxt = res.results[c]["ctxt"]              # [HPC, 65, S]
        ctx = ctxt[:, :HD, :] / ctxt[:, HD:HD + 1, :]
        # [h, hd, q] -> [q, h*HD+hd]
        out[b, :, g * EPC:(g + 1) * EPC] = ctx.transpose(2, 0, 1).reshape(S, EPC)
    return out


# revision 5
# speedup vs baseline: 1.0823x; 1.0823x over previous
"""Multi-head attention kernel for 8 TRN2 NeuronCores (v2).

Problem: B=2, S=2048, D=1024, H=16 heads (HD=64).
  q/k/v = x @ W{q,k,v}.T + b;  out = softmax(q k^T / 8 + mask) v  (heads merged)

Sharding: core c owns batch b = c // 4 and head group g = c % 4 (4 heads,
256 channels).  No collectives: each core computes its [2048, 256] slice of
the output; the host gathers.

Changes vs the 221 us baseline (TimelineSim 205 -> 172.6 us; measured
~199 us in the same session where the baseline re-measured 226 us, a
~12% speedup; all per-core PE matmul streams sit at the dataflow
roofline of 360448 moving rows = 150 us @ 2.4 GHz):
  - V bias via DVE tensor_add against a host-broadcast bias tile
    (drops the 16 ones-row bias matmuls).
  - All PSUM pools coexist (psS 2x2 banks + psPH 2x1 + psT 2x1 = 8
    banks): no mid-kernel pool-scope transition barrier (was ~6 us).
  - DMA order follows consumption order (wq, xt s-chunk 0 split in two,
    wk, biases/mask, xt 1-3, wv) so the first matmul starts ~4.5 us in.
  - One globally paced emission schedule replaces the phase windows: a
    B1 stream (the exp feed, halves-major per head) is interleaved with
    "filler" PE quanta (projection chunks, V tiles, B2 chunks) drawn
    greedily in dependency order against a rows-proportional pace, with
    per-filler eligibility (B2 chunk (h,qp,kc) unlocks 5 B1 items after
    the E halves it reads) and deadlines (K chunk c before B1 kt=4c).
    The PE stream stays dense (p-state stays at 2.4 GHz) and ACT always
    has an exp pending; B2 work drifts into the tail head's window.
  - B2 pair-chunks run kt-outer over a qc pair so each V_aug stationary
    load covers 2 matmuls (fewer PE weight reloads; measured win).

Kept from the baseline (measured ~20 us better than the "cheaper"
K=64 alternative): Q^T/K^T duplicated into both partition halves so
score matmuls contract over K=128; the 2x score is absorbed into the
exp scale (0.0625).  Real HW runs K=64 matmuls well below the cost
model's free-dim-only estimate.

Per-core dataflow (operands fp16, accumulation fp32 in PSUM):
  proj single (et, qc): psPH[128,512] = sum_d Wq/Wk[d,et]^T x^T[d,qc];
    DVE adds bias and writes per-head duplicated qt/kt [128, S] tiles.
  V quantum (st): psPH[128,256] = sum_d x^T[d,st]^T Wv[d]; DVE writes
    V_aug[st] [128, 4*65] fp16 (+bias, ones column per head).
  B1 group (h, kt, half): psS[128,1024] = K^T[kt]^T Q^T (2 matmuls),
    ACT exp(0.0625 s + mask[kt]) -> E half tile [128, 1024] fp16.
  B2 pair-chunk (h, qp, kc): psT[65, 512] (x2, qc = 2qp, 2qp+1) +=
    V_aug[kt]^T E, kt-outer over 4 kt; at kc=3 DVE copies both to SBUF
    and DMA writes ctx^T[h] (row 64 = softmax denominator; the host
    divides and transposes).
"""

import numpy as np

import concourse.mybir as mybir
import concourse.tile as tile
from concourse import bacc
from concourse.bass_utils import run_bass_kernel_spmd

B, S, D, H = 2, 2048, 1024, 16
HD = D // H           # 64
EPC = 256             # e-channels per core (4 heads)
HPC = 4               # heads per core
NDT = D // 128        # 8 d tiles
NKT = S // 128        # 16 k tiles
W = 1024              # exp width (psum tile [128, W], 2 banks)

f32 = mybir.dt.float32
fp16 = mybir.dt.float16

_CACHE = {}


def _build(reps=1, trace_sim=False, loop=False, cache=True):
    key = (reps, loop)
    if cache and key in _CACHE:
        return _CACHE[key]

    nc = bacc.Bacc("TRN2", target_bir_lowering=False, debug=False, num_devices=8)

    # xt is laid out [128, NDT*S] host-side so DMAs are d-contiguous per s-chunk
    xt_d = nc.dram_tensor("xt", [128, NDT * S], fp16, kind="ExternalInput")
    wq_d = nc.dram_tensor("wq", [128, NDT * EPC], fp16, kind="ExternalInput")
    wk_d = nc.dram_tensor("wk", [128, NDT * EPC], fp16, kind="ExternalInput")
    wv_d = nc.dram_tensor("wv", [128, NDT * EPC], fp16, kind="ExternalInput")
    bq_d = nc.dram_tensor("bq", [128, 2], f32, kind="ExternalInput")
    bk_d = nc.dram_tensor("bk", [128, 2], f32, kind="ExternalInput")
    bvb_d = nc.dram_tensor("bvb", [128, EPC], f32, kind="ExternalInput")
    mask_d = nc.dram_tensor("mask", [128, NKT], f32, kind="ExternalInput")
    ctxt_d = nc.dram_tensor("ctxt", [HPC, HD + 1, S], f32, kind="ExternalOutput")

    Exp = mybir.ActivationFunctionType.Exp

    with tile.TileContext(nc, trace_sim=trace_sim) as tc:
        with tc.tile_pool(name="sb", bufs=1) as sb, \
             tc.tile_pool(name="wpool", bufs=1) as wpool, \
             tc.tile_pool(name="xpool", bufs=1) as xpool, \
             tc.tile_pool(name="epool", bufs=52) as epool, \
             tc.tile_pool(name="cout", bufs=4) as cout, \
             tc.tile_pool(name="psS", bufs=2, space="PSUM") as psS, \
             tc.tile_pool(name="psPH", bufs=2, space="PSUM") as psPH, \
             tc.tile_pool(name="psT", bufs=2, space="PSUM") as psT:

            # ---- loads (DMA queue order == consumption order) ----
            bq_sb = sb.tile([128, 2], f32, tag="bq")
            bk_sb = sb.tile([128, 2], f32, tag="bk")
            mask_sb = sb.tile([128, NKT], f32, tag="mask")
            bvb_sb = sb.tile([128, EPC], f32, tag="bvb")
            bvb_r = bvb_sb.rearrange("p (h c) -> p h c", h=HPC)

            wq_sb, wk_sb, wv_sb = [], [], []
            bigs = {}
            for name, lst in (("wq", wq_sb), ("wk", wk_sb), ("wv", wv_sb)):
                big = wpool.tile([128, NDT * EPC], fp16, tag=name, name=name)
                bigs[name] = big
                lst.extend(big[:, d * EPC:(d + 1) * EPC] for d in range(NDT))

            xt_big = xpool.tile([128, NDT * S], fp16, tag="xt", name="xt")
            xt_r = xt_big.rearrange("p (d s) -> p d s", d=NDT)
            xtd_r = xt_d.rearrange("p (d s) -> p d s", d=NDT)
            xt_sb = [xt_big[:, d * S:(d + 1) * S] for d in range(NDT)]

            nc.sync.dma_start(out=bigs["wq"], in_=wq_d[:, :])
            nc.sync.dma_start(out=xt_r[:, 0:4, 0:512], in_=xtd_r[:, 0:4, 0:512])
            nc.sync.dma_start(out=xt_r[:, 4:8, 0:512], in_=xtd_r[:, 4:8, 0:512])
            nc.sync.dma_start(out=bigs["wk"], in_=wk_d[:, :])
            nc.sync.dma_start(out=bq_sb, in_=bq_d[:, :])
            nc.sync.dma_start(out=bk_sb, in_=bk_d[:, :])
            nc.sync.dma_start(out=mask_sb, in_=mask_d[:, :])
            nc.sync.dma_start(out=bvb_sb, in_=bvb_d[:, :])
            for sc in range(1, 4):
                nc.sync.dma_start(
                    out=xt_r[:, :, sc * 512:(sc + 1) * 512],
                    in_=xtd_r[:, :, sc * 512:(sc + 1) * 512],
                )
            nc.sync.dma_start(out=bigs["wv"], in_=wv_d[:, :])

            # Q^T/K^T per-head tiles [128, S]: head's 64 rows duplicated into
            # both partition halves (contraction 128 at full SBUF bandwidth;
            # scores double, absorbed into the exp scale)
            qt_sb = [sb.tile([128, S], fp16, tag=f"qt{h}", name=f"qt{h}")
                     for h in range(HPC)]
            kt_sb = [sb.tile([128, S], fp16, tag=f"kt{h}", name=f"kt{h}")
                     for h in range(HPC)]
            vaug = [sb.tile([128, HPC * (HD + 1)], fp16, tag=f"vaug{st}",
                            name=f"vaug{st}")
                    for st in range(NKT)]

            def emit_body(tc, rep):
                e_map = {}
                pst_map = {}

                Ident = mybir.ActivationFunctionType.Identity

                def proj_epilogue(ps, b_sb, dsts, et, qc, act_dup=False):
                    cs = slice(qc * 512, (qc + 1) * 512)
                    for i, h in enumerate((2 * et, 2 * et + 1)):
                        src = ps[i * 64:(i + 1) * 64, :]
                        bias = b_sb[i * 64:(i + 1) * 64, et:et + 1]
                        nc.vector.tensor_scalar_add(dsts[h][0:64, cs], src, bias)
                        if act_dup:
                            # ACT is idle before the first exp: do the
                            # duplicate-half write there so DVE isn't the
                            # startup critical path
                            nc.scalar.activation(
                                dsts[h][64:128, cs], src, Ident, bias=bias)
                        else:
                            nc.vector.tensor_scalar_add(
                                dsts[h][64:128, cs], src, bias)

                def proj_single(w_lst, b_sb, dsts, et, qc, act_dup=False):
                    """One qc (startup only: needs just one xt s-chunk)."""
                    ps = psPH.tile([128, 512], f32, tag="ph", name="ph")
                    for d in range(NDT):
                        nc.tensor.matmul(
                            ps,
                            w_lst[d][:, et * 128:(et + 1) * 128],
                            xt_sb[d][:, qc * 512:(qc + 1) * 512],
                            start=(d == 0), stop=(d == NDT - 1),
                        )
                    proj_epilogue(ps, b_sb, dsts, et, qc, act_dup=act_dup)

                def v_quantum(st):
                    ps = psPH.tile([128, EPC], f32, tag="ph", name="ph")
                    for d in range(NDT):
                        nc.tensor.matmul(
                            ps, xt_sb[d][:, st * 128:(st + 1) * 128], wv_sb[d],
                            start=(d == 0), stop=(d == NDT - 1),
                        )
                    vt = vaug[st].rearrange("p (h c) -> p h c", h=HPC)
                    nc.vector.memset(vt[:, :, HD:HD + 1], 1.0)
                    nc.vector.tensor_add(
                        vt[:, :, 0:HD],
                        ps.rearrange("p (h c) -> p h c", h=HPC),
                        bvb_r,
                    )

                def b1_group(h, kt, half):
                    e_t = epool.tile([128, W], fp16, tag="E", name="E")
                    e_map[(h, kt, half)] = e_t
                    ps = psS.tile([128, W], f32, tag="psS", name="psS")
                    for j in range(2):
                        qc = half * 2 + j
                        nc.tensor.matmul(
                            ps[:, j * 512:(j + 1) * 512],
                            kt_sb[h][:, kt * 128:(kt + 1) * 128],
                            qt_sb[h][:, qc * 512:(qc + 1) * 512],
                            start=True, stop=True,
                        )
                    # inputs duplicated across partition halves -> raw scores
                    # are 2x -> scale 1/(2*sqrt(HD))
                    nc.scalar.activation(
                        e_t, ps, Exp,
                        bias=mask_sb[:, kt:kt + 1], scale=0.0625,
                    )

                def b2_chunk_pair(h, qp, kc):
                    """8 matmuls: kt-outer, qc-pair-inner, so each vaug[kt]
                    stationary load covers 2 consecutive matmuls."""
                    qcs = (2 * qp, 2 * qp + 1)
                    if kc == 0:
                        for qc in qcs:
                            pst_map[(h, qc)] = psT.tile(
                                [HD + 1, 512], f32, tag="psT", name="psT")
                    for kt in range(4 * kc, 4 * kc + 4):
                        e_t = e_map[(h, kt, qp)]
                        for j, qc in enumerate(qcs):
                            nc.tensor.matmul(
                                pst_map[(h, qc)],
                                vaug[kt][:, h * (HD + 1):(h + 1) * (HD + 1)],
                                e_t[:, j * 512:(j + 1) * 512],
                                start=(kt == 0), stop=(kt == NKT - 1),
                            )
                    if kc == 3:
                        for qc in qcs:
                            o = cout.tile([HD + 1, 512], f32, tag="cout",
                                          name="cout")
                            nc.vector.tensor_copy(o, pst_map[(h, qc)])
                            nc.sync.dma_start(
                                out=ctxt_d[h, :, qc * 512:(qc + 1) * 512], in_=o)
                            del pst_map[(h, qc)]

                # ---- globally paced schedule ----
                # B1 stream items: per head, all half-0 groups then all
                # half-1 (halves-major: the first exp fires early and B2
                # qc-pairs unlock at mid-head).
                b1_items = []
                for h in range(HPC):
                    for half in range(2):
                        for kt in range(NKT):
                            b1_items.append((1024, [(h, kt, half)]))

                # Fillers: (rows, eligible_idx, deadline_idx, thunk), emitted
                # in list order.  eligible: not before b1_items[eligible] has
                # been emitted; deadline: must be emitted before
                # b1_items[deadline].
                INF = 10 ** 9
                fillers = []

                # rest of e-tile 0 as singles (startup is DMA-paced; a
                # coarser quantum would stall on two xt s-chunks at once).
                # K chunk c feeds B1 kt 4c..4c+3; Q chunks 2,3 feed the
                # half-1 items.
                def ps1(w, b, dst, et, qc, dl):
                    fillers.append((4096, 0, dl,
                                    lambda: proj_single(w, b, dst, et, qc)))

                ps1(wk_sb, bk_sb, kt_sb, 0, 1, 4)
                ps1(wk_sb, bk_sb, kt_sb, 0, 2, 8)
                ps1(wk_sb, bk_sb, kt_sb, 0, 3, 12)
                ps1(wq_sb, bq_sb, qt_sb, 0, 2, 16)
                ps1(wq_sb, bq_sb, qt_sb, 0, 3, 16)
                # V (list order keeps all V before the first B2 chunk)
                for st in range(NKT):
                    fillers.append((2048, 0, 17 + st,
                                    lambda st=st: v_quantum(st)))
                # B2 h0 pair-chunks: E halves (0, kt<=4kc+3, qp) ready after
                # b1 item 16*qp + 4kc + 3
                for qp in range(2):
                    for kc in range(4):
                        el = 16 * qp + 4 * kc + 4
                        fillers.append(
                            (4096, el, INF,
                             lambda qp=qp, kc=kc: b2_chunk_pair(0, qp, kc)))
                # e-tile 1 (heads 2, 3): needed progressively by B1 h2
                # (items 64-95); singles keep the exp cadence smooth
                ps1(wq_sb, bq_sb, qt_sb, 1, 0, 64)
                ps1(wk_sb, bk_sb, kt_sb, 1, 0, 64)
                ps1(wq_sb, bq_sb, qt_sb, 1, 1, 64)
                ps1(wk_sb, bk_sb, kt_sb, 1, 1, 68)
                ps1(wk_sb, bk_sb, kt_sb, 1, 2, 72)
                ps1(wk_sb, bk_sb, kt_sb, 1, 3, 76)
                ps1(wq_sb, bq_sb, qt_sb, 1, 2, 80)
                ps1(wq_sb, bq_sb, qt_sb, 1, 3, 80)
                # B2 h1-3: chunk (h, qp, kc) eligible after b1 item
                # 32h + 16*qp + 4kc + 3
                for h in (1, 2, 3):
                    for qp in range(2):
                        for kc in range(4):
                            el = 32 * h + 16 * qp + 4 * kc + 4
                            fillers.append(
                                (4096, el, INF,
                                 lambda h=h, qp=qp, kc=kc: b2_chunk_pair(h, qp, kc)))

                # prologue: enough of Q/K e-tile 0 for B1 h0 to start, in
                # single-qc quanta so the first matmul only needs xt chunk 0
                # (act_dup=True would do the duplicate-half writes on the
                # then-idle ACT engine — ~1 us better in TimelineSim, but on
                # HW the Identity<->Exp activation-table switch is not free,
                # so it measured neutral-to-worse; keep DVE.)
                proj_single(wq_sb, bq_sb, qt_sb, 0, 0)
                proj_single(wk_sb, bk_sb, kt_sb, 0, 0)
                proj_single(wq_sb, bq_sb, qt_sb, 0, 1)

                total_rows = sum(f[0] for f in fillers)
                b1_rows = sum(r for r, _ in b1_items)
                ratio = total_rows / b1_rows
                acc = 0.0
                fi = 0
                for i, (rows, groups) in enumerate(b1_items):
                    while fi < len(fillers) and fillers[fi][2] <= i:
                        fillers[fi][3]()
                        acc -= fillers[fi][0]
                        fi += 1
                    acc += rows * ratio
                    while (fi < len(fillers) and fillers[fi][1] <= i
                           and fillers[fi][0] <= acc):
                        fillers[fi][3]()
                        acc -= fillers[fi][0]
                        fi += 1
                    for h, kt, half in groups:
                        b1_group(h, kt, half)
                while fi < len(fillers):
                    fillers[fi][3]()
                    fi += 1

            if loop and reps > 1:
                with tc.For_i(0, reps, 1):
                    emit_body(tc, 0)
            else:
                for rep in range(reps):
                    emit_body(tc, rep)

    nc.compile()
    if cache:
        _CACHE[key] = nc
    return nc


def _wlayout(w):
    """[EPC, D] nn.Linear weight slice -> [128, NDT*EPC] (d-tiles along free dim)."""
    wt = w.T.astype(np.float16).reshape(NDT, 128, EPC)
    return np.ascontiguousarray(wt.transpose(1, 0, 2).reshape(128, NDT * EPC))


def _prep_inputs(hidden_states, attn_mask, Wq, bq, Wk, bk, Wv, bv):
    """Build the 8 per-core input maps (host-side sharding)."""
    in_maps = []
    xt_b = {}
    for b in range(B):
        xt = hidden_states[b].T.astype(np.float16).reshape(NDT, 128, S)  # [D, S] tiled
        xt_b[b] = np.ascontiguousarray(xt.transpose(1, 0, 2).reshape(128, NDT * S))
    mask_b = {
        b: np.ascontiguousarray(
            np.asarray(attn_mask[b, 0, 0, :], dtype=np.float32).reshape(NKT, 128).T
        )
        for b in range(B)
    }
    for c in range(8):
        b, g = divmod(c, HPC)
        sl = slice(g * EPC, (g + 1) * EPC)
        in_maps.append({
            "xt": xt_b[b],
            "wq": _wlayout(Wq[sl, :]),
            "wk": _wlayout(Wk[sl, :]),
            "wv": _wlayout(Wv[sl, :]),
            "bq": np.ascontiguousarray(np.asarray(bq[sl], np.float32).reshape(2, 128).T),
            "bk": np.ascontiguousarray(np.asarray(bk[sl], np.float32).reshape(2, 128).T),
            "bvb": np.ascontiguousarray(
                np.tile(np.asarray(bv[sl], np.float32).reshape(1, EPC), (128, 1))),
            "mask": mask_b[b],
        })
    return in_maps


def kernel(hidden_states, attn_mask, Wq, bq, Wk, bk, Wv, bv):
    hidden_states = np.asarray(hidden_states)
    attn_mask = np.asarray(attn_mask)
    Wq, bq = np.asarray(Wq), np.asarray(bq)
    Wk, bk = np.asarray(Wk), np.asarray(bk)
    Wv, bv = np.asarray(Wv), np.asarray(bv)

    nc = _build()
    in_maps = _prep_inputs(hidden_states, attn_mask, Wq, bq, Wk, bk, Wv, bv)
    res = run_bass_kernel_spmd(nc, in_maps, list(range(8)))

    out = np.empty((B, S, D), np.float32)
    for c in range(8):
        b, g = divmod(c, HPC)
        ctxt = res.results[c]["ctxt"]              # [HPC, 65, S]
        ctx = ctxt[:, :HD, :] / ctxt[:, HD:HD + 1, :]
        # [h, hd, q] -> [q, h*HD+hd]
        out[b, :, g * EPC:(g + 1) * EPC] = ctx.transpose(2, 0, 1).reshape(S, EPC)
    return out
